# revision 1
# baseline (speedup 1.0000x reference)
"""Trainium2 Bass kernel for LowRankRayTracer.

csi[f] = (delta_t/D) * v_f^T M v_f,  M = conj(rad)^T conj(att)  (R=32, complex)
contracted over N = D*K = 524288 rows.

Strategy (8 cores):
  - Shard the N rows across cores (512 directions each). csi is linear in M,
    so each core computes its partial S = rad32^T att32 (64x64, f32 view of
    complex pairs -> all four real cross products at once), builds
    W = [W_real | W_imag] (block form), computes partial csi over ALL F=8192
    subcarriers, and the host just sums the 8 partial csi vectors.
  - fp32 matmul is 4 cyc/col on TRN2 PE, so inputs are split on the host into
    fp16 hi+lo (same total bytes); with the att hi/lo pair packed side by side
    as one 256-wide moving operand, two matmuls per slice (lhsT=rad_h, rad_l)
    produce all four products hh|hl|lh|ll -- exact reconstruction, and half
    the LDWEIGHTS of a 3-pass version (LDW is the PE bottleneck otherwise).
  - Matmuls accumulate round-robin into 4 PSUM banks (avoids same-bank RMW
    serialization); diagonal blocks summed later via selection matmuls.
"""

import numpy as np

D, K, R = 4096, 128, 32
F = 8192
N_CORES = 8
DIR_PER_CORE = D // N_CORES              # 512
ROWS_PER_CORE = DIR_PER_CORE * K         # 65536 rows of (64,) f32
N_MACRO = 8                              # macro tiles per tensor per core
MACRO_COLS = 4096                        # fp16 per partition per macro tile
SLICE = 128                              # matmul slice width (2 rows/partition)
SCALE = (200.0 / K) / D                  # delta_t / num_directions (exact binary)
FCHUNK = 512                             # phase-3 subcarriers per chunk
N_FCHUNK = F // FCHUNK                   # 16
NB = 4                                   # round-robin PSUM accumulator banks

_NC_CACHE = {}


def _build_consts():
    """(128, 258) f32: four (128,64) selection matrices + ones-selector cols."""
    c = np.zeros((128, 258), np.float32)
    EA = np.zeros((128, 32), np.float32)
    OA = np.zeros((128, 32), np.float32)
    EB = np.zeros((128, 32), np.float32)
    OB = np.zeros((128, 32), np.float32)
    for m in range(32):
        EA[2 * m, m] = 1.0
        OA[2 * m + 1, m] = 1.0
        EB[64 + 2 * m, m] = 1.0
        OB[64 + 2 * m + 1, m] = 1.0
    c[:, 0:32] = EA
    c[:, 32:64] = OA
    c[:, 64:96] = EB
    c[:, 96:128] = OB
    c[:, 128:160] = OA
    c[:, 160:192] = EA
    c[:, 192:224] = OB
    c[:, 224:256] = EB
    c[0:64, 256] = 1.0
    c[64:128, 257] = 1.0
    return c


def build_nc(n_macro=N_MACRO):
    import concourse.bacc as bacc
    import concourse.mybir as mybir
    import concourse.tile as tile

    fp32 = mybir.dt.float32
    fp16 = mybir.dt.float16
    nc = bacc.Bacc(trn_type="TRN2", target_bir_lowering=False, debug=False)

    rad_h_d = nc.dram_tensor("rad_h", [n_macro, 128, MACRO_COLS], fp16,
                             kind="ExternalInput").ap()
    rad_l_d = nc.dram_tensor("rad_l", [n_macro, 128, MACRO_COLS], fp16,
                             kind="ExternalInput").ap()
    att_hl_d = nc.dram_tensor("att_hl", [n_macro, 128, 2 * MACRO_COLS], fp16,
                              kind="ExternalInput").ap()
    gtd_d = nc.dram_tensor("gtd", [128, F], fp32, kind="ExternalInput").ap()
    gth_d = nc.dram_tensor("gth", [64, F], fp16, kind="ExternalInput").ap()
    gtl_d = nc.dram_tensor("gtl", [64, F], fp16, kind="ExternalInput").ap()
    cst_d = nc.dram_tensor("consts", [128, 258], fp32, kind="ExternalInput").ap()
    out_d = nc.dram_tensor("csi", [2, F], fp32, kind="ExternalOutput").ap()

    with tile.TileContext(nc) as tc:
        with (
            tc.tile_pool(name="io", bufs=2) as io_pool,
            tc.tile_pool(name="small", bufs=1) as small,
            tc.tile_pool(name="epool", bufs=8) as epool,
        ):
            # constants up front (tiny); gtd issued after the main-loop DMAs
            # so it doesn't steal early HBM bandwidth (not needed till phase 3)
            c_sb = small.tile([128, 258], fp32, tag="consts")
            nc.sync.dma_start(c_sb[:], cst_d[:])
            gtd_sb = small.tile([128, F], fp32, tag="gtd")
            gth_sb = small.tile([64, F], fp16, tag="gth")
            gtl_sb = small.tile([64, F], fp16, tag="gtl")

            # ---- main loop: S += rad^T att via fp16 hi/lo, 256-wide rhs ----
            # lhsT=rad_h over rhs=[att_h|att_l] gives [hh|hl]; lhsT=rad_l
            # gives [lh|ll]. S = sum of all four 128-col blocks (exact).
            s_sb = small.tile([128, 128], fp32, tag="s_sb")
            n_slices = MACRO_COLS // SLICE
            total = n_macro * n_slices * 2
            with tc.tile_pool(name="spsum", bufs=1, space="PSUM") as spsum:
                banks = [spsum.tile([128, 2 * SLICE], fp32, tag=f"s{b}",
                                    name=f"sbank{b}")
                         for b in range(NB)]
                seen = [False] * NB
                idx = 0
                for i in range(n_macro):
                    rad_h = io_pool.tile([128, MACRO_COLS], fp16, tag="rad_h")
                    rad_l = io_pool.tile([128, MACRO_COLS], fp16, tag="rad_l")
                    att_hl = io_pool.tile([128, 2 * MACRO_COLS], fp16,
                                          tag="att_hl")
                    if i == 0:
                        # halve the first loads so the first matmuls start
                        # as soon as ~1.5 MiB has landed, not 4 MiB
                        hm = MACRO_COLS // 2
                        nc.sync.dma_start(rad_h[:, 0:hm], rad_h_d[0, :, 0:hm])
                        nc.scalar.dma_start(att_hl[:, 0:2 * hm],
                                            att_hl_d[0, :, 0:2 * hm])
                        nc.sync.dma_start(rad_l[:, 0:hm], rad_l_d[0, :, 0:hm])
                        nc.sync.dma_start(rad_h[:, hm:], rad_h_d[0, :, hm:])
                        nc.scalar.dma_start(att_hl[:, 2 * hm:],
                                            att_hl_d[0, :, 2 * hm:])
                        nc.sync.dma_start(rad_l[:, hm:], rad_l_d[0, :, hm:])
                    else:
                        nc.sync.dma_start(rad_h[:], rad_h_d[i, :, :])
                        nc.sync.dma_start(rad_l[:], rad_l_d[i, :, :])
                        nc.scalar.dma_start(att_hl[:], att_hl_d[i, :, :])
                    for s in range(n_slices):
                        rsl = slice(s * SLICE, (s + 1) * SLICE)
                        asl = slice(s * 2 * SLICE, (s + 1) * 2 * SLICE)
                        for lh in (rad_h, rad_l):
                            b = idx % NB
                            nc.tensor.matmul(
                                banks[b][:],
                                lhsT=lh[:, rsl],
                                rhs=att_hl[:, asl],
                                start=not seen[b],
                                stop=(idx >= total - NB),
                            )
                            seen[b] = True
                            idx += 1

                nc.sync.dma_start(gtd_sb[:], gtd_d[:])
                nc.sync.dma_start(gth_sb[:], gth_d[:])
                nc.sync.dma_start(gtl_sb[:], gtl_d[:])

                # S = sum of all four 128-col blocks over the 4 banks
                acc = small.tile([128, 2 * SLICE], fp32, tag="acc")
                nc.vector.tensor_copy(acc[:], banks[0][:])
                for b in range(1, NB):
                    nc.vector.tensor_add(acc[:], acc[:], banks[b][:])
                nc.vector.tensor_add(s_sb[:], acc[:, 0:SLICE],
                                     acc[:, SLICE:2 * SLICE])

            # ---- epilogue: build W = [W_real | W_imag] (64, 128) ----
            with tc.tile_pool(name="vpsum", bufs=1, space="PSUM") as vpsum:
                v1 = vpsum.tile([64, 64], fp32, tag="v1")
                nc.tensor.matmul(v1[:], lhsT=c_sb[:, 0:64], rhs=s_sb[:, 0:64],
                                 start=True, stop=False)
                nc.tensor.matmul(v1[:], lhsT=c_sb[:, 64:128],
                                 rhs=s_sb[:, 64:128], start=False, stop=True)
                v2 = vpsum.tile([64, 64], fp32, tag="v2")
                nc.tensor.matmul(v2[:], lhsT=c_sb[:, 128:192],
                                 rhs=s_sb[:, 0:64], start=True, stop=False)
                nc.tensor.matmul(v2[:], lhsT=c_sb[:, 192:256],
                                 rhs=s_sb[:, 64:128], start=False, stop=True)

                v1s = small.tile([64, 64], fp32, tag="v1s")
                nc.vector.tensor_copy(v1s[:], v1[:])
                v2s = small.tile([64, 64], fp32, tag="v2s")
                nc.vector.tensor_copy(v2s[:], v2[:])

            # mr = Mr (dup-stacked), mp = -Mi (dup-stacked)
            mr = small.tile([64, 32], fp32, tag="mr")
            mp = small.tile([64, 32], fp32, tag="mp")
            nc.vector.tensor_sub(mr[0:32, :], v1s[0:32, 0:64:2], v2s[0:32, 1:64:2])
            nc.vector.tensor_sub(mr[32:64, :], v2s[32:64, 0:64:2], v1s[32:64, 1:64:2])
            nc.vector.tensor_add(mp[0:32, :], v1s[0:32, 1:64:2], v2s[0:32, 0:64:2])
            nc.vector.tensor_add(mp[32:64, :], v2s[32:64, 1:64:2], v1s[32:64, 0:64:2])

            wri = small.tile([64, 128], fp32, tag="wri")
            s_ = float(SCALE)
            # W_real = [[Mr, -Mi], [-Mi, -Mr]] * s
            nc.scalar.mul(wri[0:32, 0:32], mr[0:32, :], s_)
            nc.scalar.mul(wri[0:32, 32:64], mp[0:32, :], s_)
            nc.scalar.mul(wri[32:64, 0:32], mp[32:64, :], s_)
            nc.scalar.mul(wri[32:64, 32:64], mr[32:64, :], -s_)
            # W_imag = [[Mi, Mr], [Mr, -Mi]] * s
            nc.scalar.mul(wri[0:32, 64:96], mp[0:32, :], -s_)
            nc.scalar.mul(wri[0:32, 96:128], mr[0:32, :], s_)
            nc.scalar.mul(wri[32:64, 64:96], mr[32:64, :], s_)
            nc.scalar.mul(wri[32:64, 96:128], mp[32:64, :], s_)

            # fp16 hi/lo split of W for the phase-3 matmuls
            wh = small.tile([64, 128], fp16, tag="wh")
            nc.vector.tensor_copy(wh[:], wri[:])
            whf = small.tile([64, 128], fp32, tag="whf")
            nc.vector.tensor_copy(whf[:], wh[:])
            wlf = small.tile([64, 128], fp32, tag="wlf")
            nc.vector.tensor_sub(wlf[:], wri[:], whf[:])
            wl = small.tile([64, 128], fp16, tag="wl")
            nc.vector.tensor_copy(wl[:], wlf[:])

            # PE warm-keepers: cheap matmuls dependent on s_sb bridge the
            # epilogue gap so HAM doesn't re-throttle before phase 3
            with tc.tile_pool(name="wpsum", bufs=1, space="PSUM") as wpsum:
                warm_ps = wpsum.tile([64, 64], fp32, tag="warm")
                for w in range(10):
                    nc.tensor.matmul(warm_ps[:], lhsT=c_sb[:, 0:64],
                                     rhs=s_sb[:, 0:64], start=True, stop=True)

            # ---- phase 3: csi chunks over F ----
            # All T matmuls issued first so the per-chunk csi matmuls don't
            # head-of-line-block them in the in-order PE queue.
            csi_sb = small.tile([2, F], fp32, tag="csi_sb")
            with (
                tc.tile_pool(name="tpsum", bufs=6, space="PSUM") as tpsum,
                tc.tile_pool(name="cpsum", bufs=2, space="PSUM") as cpsum,
            ):
                t_tiles = []
                e_tiles = []
                for ci in range(N_FCHUNK):
                    fs = slice(ci * FCHUNK, (ci + 1) * FCHUNK)
                    t_ps = tpsum.tile([128, FCHUNK], fp32, tag="t",
                                      name=f"t{ci}")
                    # T = W^T g via fp16 hi/lo (dropped Wl*gl ~ 2^-22)
                    nc.tensor.matmul(t_ps[:], lhsT=wh[:], rhs=gth_sb[:, fs],
                                     start=True, stop=False)
                    nc.tensor.matmul(t_ps[:], lhsT=wl[:], rhs=gth_sb[:, fs],
                                     start=False, stop=False)
                    nc.tensor.matmul(t_ps[:], lhsT=wh[:], rhs=gtl_sb[:, fs],
                                     start=False, stop=True)
                    t_tiles.append(t_ps)
                    e_sb = epool.tile([128, FCHUNK], fp32, tag="e",
                                      name=f"e{ci}")
                    nc.vector.tensor_mul(e_sb[:], gtd_sb[:, fs], t_ps[:])
                    e_tiles.append(e_sb)
                for ci in range(N_FCHUNK):
                    fs = slice(ci * FCHUNK, (ci + 1) * FCHUNK)
                    c_ps = cpsum.tile([2, FCHUNK], fp32, tag="c",
                                      name=f"c{ci}")
                    nc.tensor.matmul(c_ps[:], lhsT=c_sb[:, 256:258],
                                     rhs=e_tiles[ci][:], start=True, stop=True)
                    nc.scalar.copy(csi_sb[:, fs], c_ps[:])

            nc.sync.dma_start(out_d[:], csi_sb[:])

    nc.compile()
    return nc


def _prep_shared(fbv):
    """gtd (128,F) f32 dup + fp16 hi/lo (64,F) from complex fbv (F, R)."""
    fbv32 = np.ascontiguousarray(fbv).view(np.float32).reshape(F, 2 * R)
    gbt = np.ascontiguousarray(
        np.concatenate([fbv32[:, 0::2].T, fbv32[:, 1::2].T], axis=0))
    gtd = np.ascontiguousarray(np.concatenate([gbt, gbt], axis=0))
    gth = gbt.astype(np.float16)
    gtl = (gbt - gth.astype(np.float32)).astype(np.float16)
    return gtd, gth, gtl


def _shard_hl(arr, core):
    """Core's complex64 shard -> (hi, lo) fp16 arrays (N_MACRO,128,MACRO_COLS)."""
    sh = arr[core * DIR_PER_CORE:(core + 1) * DIR_PER_CORE]
    f32 = np.ascontiguousarray(sh).view(np.float32).ravel()
    h = f32.astype(np.float16)
    lo = (f32 - h.astype(np.float32)).astype(np.float16)
    shp = (N_MACRO, 128, MACRO_COLS)
    return h.reshape(shp), lo.reshape(shp)


def _pack_hl(h, lo):
    """Interleave hi/lo at 128-col slice granularity: [...,s*256:+256] =
    [h_slice(128) | lo_slice(128)] -> (N_MACRO, 128, 2*MACRO_COLS)."""
    ns = MACRO_COLS // SLICE
    h4 = h.reshape(N_MACRO, 128, ns, SLICE)
    l4 = lo.reshape(N_MACRO, 128, ns, SLICE)
    return np.ascontiguousarray(
        np.stack([h4, l4], axis=3).reshape(N_MACRO, 128, 2 * MACRO_COLS))


def kernel(attenuation_vectors, radiation_vectors, frequency_basis_vectors):
    from concourse.bass_utils import run_bass_kernel_spmd

    if "nc" not in _NC_CACHE:
        _NC_CACHE["nc"] = build_nc()
    nc = _NC_CACHE["nc"]

    gtd, gth, gtl = _prep_shared(frequency_basis_vectors)
    consts = _build_consts()
    in_maps = []
    for c in range(N_CORES):
        rh, rl = _shard_hl(radiation_vectors, c)
        ah, al = _shard_hl(attenuation_vectors, c)
        in_maps.append({
            "rad_h": rh, "rad_l": rl,
            "att_hl": _pack_hl(ah, al),
            "gtd": gtd, "gth": gth, "gtl": gtl,
            "consts": consts,
        })

    res = run_bass_kernel_spmd(nc, in_maps, core_ids=list(range(N_CORES)))
    acc = np.zeros((2, F), np.float64)
    for r in res.results:
        acc += r["csi"]
    return (acc[0] + 1j * acc[1]).astype(np.complex64)



# revision 3
# speedup vs baseline: 1.6345x; 1.6345x over previous
"""Trainium2 Bass kernel for LowRankRayTracer.

csi[f] = (delta_t/D) * v_f^T M v_f,  M = conj(rad)^T conj(att)  (R=32, complex)
contracted over N = D*K = 524288 rows.

Strategy (8 cores):
  - Shard the N rows across cores (512 directions each). csi is linear in M,
    so each core computes its partial S = rad^T att (128x128, f32 view of
    complex pairs + 2-rows-per-partition packing), builds W = [W_real|W_imag],
    computes partial csi over ALL F=8192 subcarriers, and the host just sums
    the 8 partial csi vectors.
  - Precision budget: the harness gate is rel_err < 2e-2 and fp16-quantized
    inputs give ~1e-3, so rad/att/g/W/e are all streamed as fp16 "hi" only
    (no lo-correction passes). This halves HBM traffic vs fp32 bytes and
    quarters the main-loop PE columns vs the hi/lo exact version.
  - Matmuls accumulate round-robin into 4 bank-sized PSUM tiles (avoids
    same-bank RMW serialization); banks + row-pair diagonal summed later
    (vector adds + selection matmuls).
"""

import numpy as np

D, K, R = 4096, 128, 32
F = 8192
N_CORES = 8
DIR_PER_CORE = D // N_CORES              # 512
N_MACRO = 8                              # macro tiles per tensor per core
MACRO_COLS = 4096                        # fp16 per partition per macro tile
SLICE = 128                              # matmul slice width (2 rows/partition)
SCALE = (200.0 / K) / D                  # delta_t / num_directions (exact binary)
FCHUNK = 512                             # phase-3 subcarriers per chunk
N_FCHUNK = F // FCHUNK                   # 16
NB = 4                                   # round-robin PSUM accumulator banks

_NC_CACHE = {}


def _build_consts():
    """(128, 258) f32: four (128,64) selection matrices + ones-selector cols."""
    c = np.zeros((128, 258), np.float32)
    EA = np.zeros((128, 32), np.float32)
    OA = np.zeros((128, 32), np.float32)
    EB = np.zeros((128, 32), np.float32)
    OB = np.zeros((128, 32), np.float32)
    for m in range(32):
        EA[2 * m, m] = 1.0
        OA[2 * m + 1, m] = 1.0
        EB[64 + 2 * m, m] = 1.0
        OB[64 + 2 * m + 1, m] = 1.0
    c[:, 0:32] = EA
    c[:, 32:64] = OA
    c[:, 64:96] = EB
    c[:, 96:128] = OB
    c[:, 128:160] = OA
    c[:, 160:192] = EA
    c[:, 192:224] = OB
    c[:, 224:256] = EB
    c[0:64, 256] = 1.0
    c[64:128, 257] = 1.0
    return c


def build_nc(n_macro=N_MACRO):
    import concourse.bacc as bacc
    import concourse.mybir as mybir
    import concourse.tile as tile

    fp32 = mybir.dt.float32
    fp16 = mybir.dt.float16
    nc = bacc.Bacc(trn_type="TRN2", target_bir_lowering=False, debug=False)

    rad_d = nc.dram_tensor("rad_h", [n_macro, 128, MACRO_COLS], fp16,
                           kind="ExternalInput").ap()
    att_d = nc.dram_tensor("att_h", [n_macro, 128, MACRO_COLS], fp16,
                           kind="ExternalInput").ap()
    gth_d = nc.dram_tensor("gth", [64, F], fp16, kind="ExternalInput").ap()
    cst_d = nc.dram_tensor("consts", [128, 258], fp32, kind="ExternalInput").ap()
    out_d = nc.dram_tensor("csi", [2, F], fp32, kind="ExternalOutput").ap()

    with tile.TileContext(nc) as tc:
        with (
            tc.tile_pool(name="io", bufs=2) as io_pool,
            tc.tile_pool(name="small", bufs=1) as small,
            tc.tile_pool(name="epool", bufs=8) as epool,
        ):
            # constants up front (tiny); g loads issued after the main-loop
            # DMAs so they don't steal early HBM bandwidth (phase-3 only)
            c_sb = small.tile([128, 258], fp32, tag="consts")
            nc.sync.dma_start(c_sb[:], cst_d[:])
            g2_sb = small.tile([128, F], fp16, tag="g2")

            # ---- main loop: S += rad^T att, fp16 hi-only ----
            s_sb = small.tile([128, 128], fp32, tag="s_sb")
            n_slices = MACRO_COLS // SLICE
            total = n_macro * n_slices
            with tc.tile_pool(name="spsum", bufs=1, space="PSUM") as spsum:
                banks = [spsum.tile([128, 512], fp32, tag=f"s{b}",
                                    name=f"sbank{b}")
                         for b in range(NB)]
                seen = [False] * NB
                idx = 0
                for i in range(n_macro):
                    rad = io_pool.tile([128, MACRO_COLS], fp16, tag="rad")
                    att = io_pool.tile([128, MACRO_COLS], fp16, tag="att")
                    if i == 0:
                        # quarter the first loads so the first matmuls start
                        # as soon as ~1 MiB has landed, not 4 MiB
                        qm = MACRO_COLS // 4
                        for q in range(4):
                            qs = slice(q * qm, (q + 1) * qm)
                            nc.sync.dma_start(rad[:, qs], rad_d[0, :, qs])
                            nc.scalar.dma_start(att[:, qs], att_d[0, :, qs])
                    else:
                        nc.sync.dma_start(rad[:], rad_d[i, :, :])
                        nc.scalar.dma_start(att[:], att_d[i, :, :])
                    for s in range(n_slices):
                        sl = slice(s * SLICE, (s + 1) * SLICE)
                        b = idx % NB
                        nc.tensor.matmul(
                            banks[b][:, 0:SLICE],
                            lhsT=rad[:, sl],
                            rhs=att[:, sl],
                            start=not seen[b],
                            stop=(idx >= total - NB),
                        )
                        seen[b] = True
                        idx += 1

                # duplicated g (fp16) on 128 partitions: two HBM reads of the
                # same [64, F] tensor into the upper/lower partition halves
                nc.sync.dma_start(g2_sb[0:64, :], gth_d[:])
                nc.sync.dma_start(g2_sb[64:128, :], gth_d[:])

                # S = sum of the 4 round-robin banks (DVE may read at most
                # one PSUM operand per instruction)
                nc.vector.tensor_copy(s_sb[:], banks[0][:, 0:SLICE])
                for b in range(1, NB):
                    nc.vector.tensor_add(s_sb[:], s_sb[:], banks[b][:, 0:SLICE])

            # ---- epilogue: build W = [W_real | W_imag] (64, 128) ----
            with tc.tile_pool(name="vpsum", bufs=1, space="PSUM") as vpsum:
                v1 = vpsum.tile([64, 64], fp32, tag="v1")
                nc.tensor.matmul(v1[:], lhsT=c_sb[:, 0:64], rhs=s_sb[:, 0:64],
                                 start=True, stop=False)
                nc.tensor.matmul(v1[:], lhsT=c_sb[:, 64:128],
                                 rhs=s_sb[:, 64:128], start=False, stop=True)
                v2 = vpsum.tile([64, 64], fp32, tag="v2")
                nc.tensor.matmul(v2[:], lhsT=c_sb[:, 128:192],
                                 rhs=s_sb[:, 0:64], start=True, stop=False)
                nc.tensor.matmul(v2[:], lhsT=c_sb[:, 192:256],
                                 rhs=s_sb[:, 64:128], start=False, stop=True)

                v1s = small.tile([64, 64], fp32, tag="v1s")
                nc.vector.tensor_copy(v1s[:], v1[:])
                v2s = small.tile([64, 64], fp32, tag="v2s")
                nc.vector.tensor_copy(v2s[:], v2[:])

            # mr = Mr (dup-stacked), mp = -Mi (dup-stacked)
            mr = small.tile([64, 32], fp32, tag="mr")
            mp = small.tile([64, 32], fp32, tag="mp")
            nc.vector.tensor_sub(mr[0:32, :], v1s[0:32, 0:64:2], v2s[0:32, 1:64:2])
            nc.vector.tensor_sub(mr[32:64, :], v2s[32:64, 0:64:2], v1s[32:64, 1:64:2])
            nc.vector.tensor_add(mp[0:32, :], v1s[0:32, 1:64:2], v2s[0:32, 0:64:2])
            nc.vector.tensor_add(mp[32:64, :], v2s[32:64, 1:64:2], v1s[32:64, 0:64:2])

            wri = small.tile([64, 128], fp32, tag="wri")
            s_ = float(SCALE)
            # W_real = [[Mr, -Mi], [-Mi, -Mr]] * s
            nc.scalar.mul(wri[0:32, 0:32], mr[0:32, :], s_)
            nc.scalar.mul(wri[0:32, 32:64], mp[0:32, :], s_)
            nc.scalar.mul(wri[32:64, 0:32], mp[32:64, :], s_)
            nc.scalar.mul(wri[32:64, 32:64], mr[32:64, :], -s_)
            # W_imag = [[Mi, Mr], [Mr, -Mi]] * s
            nc.scalar.mul(wri[0:32, 64:96], mp[0:32, :], -s_)
            nc.scalar.mul(wri[0:32, 96:128], mr[0:32, :], s_)
            nc.scalar.mul(wri[32:64, 64:96], mr[32:64, :], s_)
            nc.scalar.mul(wri[32:64, 96:128], mp[32:64, :], s_)

            # fp16 W for the phase-3 matmuls; fp16 ones-selector for the
            # csi reduction (matmul operands must both be non-fp32)
            wh = small.tile([64, 128], fp16, tag="wh")
            nc.vector.tensor_copy(wh[:], wri[:])
            sel16 = small.tile([128, 2], fp16, tag="sel16")
            nc.vector.tensor_copy(sel16[:], c_sb[:, 256:258])

            # PE warm-keepers: cheap matmuls dependent on s_sb bridge the
            # epilogue gap so HAM doesn't re-throttle before phase 3
            with tc.tile_pool(name="wpsum", bufs=1, space="PSUM") as wpsum:
                warm_ps = wpsum.tile([64, 64], fp32, tag="warm")
                for w in range(10):
                    nc.tensor.matmul(warm_ps[:], lhsT=c_sb[:, 0:64],
                                     rhs=s_sb[:, 0:64], start=True, stop=True)

            # ---- phase 3: csi chunks over F ----
            # All T matmuls issued first so the per-chunk csi matmuls don't
            # head-of-line-block them in the in-order PE queue.
            csi_sb = small.tile([2, F], fp32, tag="csi_sb")
            with (
                tc.tile_pool(name="tpsum", bufs=6, space="PSUM") as tpsum,
                tc.tile_pool(name="cpsum", bufs=2, space="PSUM") as cpsum,
            ):
                e_tiles = []
                for ci in range(N_FCHUNK):
                    fs = slice(ci * FCHUNK, (ci + 1) * FCHUNK)
                    t_ps = tpsum.tile([128, FCHUNK], fp32, tag="t",
                                      name=f"t{ci}")
                    # T = W^T g, fp16 single pass
                    nc.tensor.matmul(t_ps[:], lhsT=wh[:], rhs=g2_sb[0:64, fs],
                                     start=True, stop=True)
                    e_sb = epool.tile([128, FCHUNK], fp16, tag="e",
                                      name=f"e{ci}")
                    nc.vector.tensor_mul(e_sb[:], g2_sb[:, fs], t_ps[:])
                    e_tiles.append(e_sb)
                for ci in range(N_FCHUNK):
                    fs = slice(ci * FCHUNK, (ci + 1) * FCHUNK)
                    c_ps = cpsum.tile([2, FCHUNK], fp32, tag="c",
                                      name=f"c{ci}")
                    nc.tensor.matmul(c_ps[:], lhsT=sel16[:],
                                     rhs=e_tiles[ci][:], start=True, stop=True)
                    nc.scalar.copy(csi_sb[:, fs], c_ps[:])

            nc.sync.dma_start(out_d[:], csi_sb[:])

    nc.compile()
    return nc


def _prep_shared(fbv):
    """gth (64, F) fp16 from complex fbv (F, R): rows = [Re ranks; Im ranks]."""
    fbv32 = np.ascontiguousarray(fbv).view(np.float32).reshape(F, 2 * R)
    gbt = np.ascontiguousarray(
        np.concatenate([fbv32[:, 0::2].T, fbv32[:, 1::2].T], axis=0))
    return gbt.astype(np.float16)


def _shard_h(arr, core):
    """Core's complex64 shard -> fp16 hi array (N_MACRO, 128, MACRO_COLS)."""
    sh = arr[core * DIR_PER_CORE:(core + 1) * DIR_PER_CORE]
    f32 = np.ascontiguousarray(sh).view(np.float32).ravel()
    return f32.astype(np.float16).reshape(N_MACRO, 128, MACRO_COLS)


def kernel(attenuation_vectors, radiation_vectors, frequency_basis_vectors):
    from concourse.bass_utils import run_bass_kernel_spmd

    if "nc" not in _NC_CACHE:
        _NC_CACHE["nc"] = build_nc()
    nc = _NC_CACHE["nc"]

    gth = _prep_shared(frequency_basis_vectors)
    consts = _build_consts()
    in_maps = []
    for c in range(N_CORES):
        in_maps.append({
            "rad_h": _shard_h(radiation_vectors, c),
            "att_h": _shard_h(attenuation_vectors, c),
            "gth": gth,
            "consts": consts,
        })

    res = run_bass_kernel_spmd(nc, in_maps, core_ids=list(range(N_CORES)))
    acc = np.zeros((2, F), np.float64)
    for r in res.results:
        acc += r["csi"]
    return (acc[0] + 1j * acc[1]).astype(np.complex64)
